# revision 7
# baseline (speedup 1.0000x reference)
"""Linear attention ("Transformers are RNNs") on 8 Trainium2 NeuronCores.

Problem: N=8, L=S=8192, H=8, D=Dv=32, f32.
    phi(x) = elu(x)+1
    A[d,v] = sum_s phi(K)[s,d] V[s,v]        (the /v_length ... *v_length cancels exactly)
    b[d]   = sum_s phi(K)[s,d]
    out[l,v] = (sum_d phi(Q)[l,d] A[d,v]) / (sum_d phi(Q)[l,d] b[d] + EPS)

Sharding: batch element n -> core n (fully independent, no collectives).

Device design (v2):
  - Q pre-transposed on host to [H*D, L]: contraction dim d on SBUF
    partitions, contiguous DMA, no on-device transposes.
  - phi(x) = min(exp(x), 1 + relu(x))  (exactly elu(x)+1), split across
    engines:  e = Exp(x) bf16 (ScalarE);  t = (x max 0)+1 bf16 (GpSimd
    dual-op tensor_scalar);  phi = min(e, t) bf16 (VectorE tt, 2x mode).
  - Phase 1 (64 s-subtiles of 128): per 4-head group g one bf16 matmul
        lhsT = phi(K)[:, g]  [s=128, (j,d)=128]
        rhs  = [V_g | ones]  [s=128, 129]
    accumulated in PSUM[128,129] per group.  Diagonal 32x32 j-blocks of
    cols 0:128 are A_h; col 128 is b_h.
  - Phase 1.5: assemble per group: block-diag A  [128,128] bf16 and
    block-diag b columns [128,4] bf16.
  - Phase 2 (64 l-subtiles): per group two matmuls sharing the same
    stationary phiQ^T slice: numer (N=128, lands directly in output
    layout) and den (N=4, batched per 8-subtile macro in one PSUM bank so
    the eps-add + reciprocal amortize).  Normalize with one broadcast
    tensor_tensor multiply per 2 subtiles, DMA out [l, h*32+v] f32.
"""

import sys

for _p in ("/opt/trn_rl_repo",):
    if _p not in sys.path:
        sys.path.insert(0, _p)

import numpy as np

from concourse import bacc, bass, mybir, tile
from concourse.bass_utils import run_bass_kernel_spmd

# ---------------------------------------------------------------- constants
N_BATCH = 8
L = 8192
S = 8192
H = 8
D = 32
HD = H * D  # 256
P = 128
EPS = 1e-6

F32 = mybir.dt.float32
BF16 = mybir.dt.bfloat16
AF = mybir.ActivationFunctionType
OP = mybir.AluOpType

MACRO = 8  # 128-row s-subtiles per phase-1 macro tile
N_MACRO = S // (P * MACRO)  # 8
QMACRO = 8  # l-subtiles per phase-2 macro
N_QMACRO = L // (P * QMACRO)  # 8

G = 2  # head groups (4 heads each)
VA = P + 1  # 129: V group columns + ones column


def _bcast_last(ap, n):
    """Append a stride-0 dim of size n to an AP (free-dim broadcast)."""
    ap = ap.unsqueeze(ap.ndim)
    return ap.broadcast_to(tuple(ap.shape[:-1]) + (n,))


def _phi(nc, pool, x, fd):
    """phi(x) = elu(x)+1 = min(exp(x), 1 + relu(x)); x is [P, fd] f32 SBUF.

    Returns bf16. Engine split: ACT does exp, GpSimd the relu+1, DVE the min."""
    e = pool.tile([P, fd], BF16, tag="phi_e")
    t = pool.tile([P, fd], BF16, tag="phi_t")
    phi = pool.tile([P, fd], BF16, tag="phi_o")
    nc.scalar.activation(e[:], x[:], AF.Exp)
    nc.gpsimd.tensor_scalar(t[:], x[:], 0.0, 1.0, OP.max, OP.add)
    nc.vector.tensor_tensor(phi[:], e[:], t[:], OP.min)
    return phi


def _build_body(nc, tc, qt, kk, vv, out):
    with (
        tc.tile_pool(name="io", bufs=3) as io,
        tc.tile_pool(name="ew", bufs=2) as ew,
        tc.tile_pool(name="misc", bufs=1) as misc,
        tc.tile_pool(name="small", bufs=3) as small,
        tc.tile_pool(name="outp", bufs=3) as outp,
    ):
        # ---------------- phase 1: A/b accumulation over S ----------------
        with tc.tile_pool(name="ps1", bufs=1, space="PSUM") as ps1:
            pacc = [
                ps1.tile([P, VA], F32, tag=f"pacc{g}", name=f"pacc{g}")
                for g in range(G)
            ]

            for m in range(N_MACRO):
                k_t = io.tile([P, MACRO * HD], F32, tag="k_t")
                v_t = io.tile([P, MACRO * HD], F32, tag="v_t")
                rows = slice(m * MACRO * P, (m + 1) * MACRO * P)
                nc.sync.dma_start(
                    k_t[:].rearrange("p (b c) -> p b c", b=MACRO),
                    kk[rows, :].rearrange("(b p) c -> p b c", p=P),
                )
                nc.sync.dma_start(
                    v_t[:].rearrange("p (b c) -> p b c", b=MACRO),
                    vv[rows, :].rearrange("(b p) c -> p b c", p=P),
                )
                # v_r: per (subtile, group): [V_g | ones], bf16
                v_r = io.tile([P, MACRO * G * VA], BF16, tag="v_r")
                v4 = v_r[:].rearrange("p (b g c) -> p b g c", b=MACRO, g=G)
                nc.scalar.copy(
                    v4[:, :, :, 0:P],
                    v_t[:].rearrange("p (b g c) -> p b g c", b=MACRO, g=G),
                )
                nc.gpsimd.memset(v4[:, :, :, P : P + 1], 1.0)

                phi = _phi(nc, ew, k_t, MACRO * HD)

                for b in range(MACRO):
                    for g in range(G):
                        nc.tensor.matmul(
                            pacc[g][:],
                            phi[:, b * HD + g * P : b * HD + (g + 1) * P],
                            v_r[:, (b * G + g) * VA : (b * G + g + 1) * VA],
                            start=(m == 0 and b == 0),
                            stop=(m == N_MACRO - 1 and b == MACRO - 1),
                        )

            # ------------- phase 1.5: block-diag A, block-diag b ----------
            amat = []
            bmat = []
            for g in range(G):
                ag = misc.tile([P, P], BF16, tag=f"amat{g}", name=f"amat{g}")
                bg = misc.tile([P, 4], BF16, tag=f"bmat{g}", name=f"bmat{g}")
                nc.vector.memset(ag[:], 0.0)
                nc.vector.memset(bg[:], 0.0)
                for j in range(4):
                    r0 = 32 * j
                    nc.scalar.copy(
                        ag[r0 : r0 + 32, r0 : r0 + 32],
                        pacc[g][r0 : r0 + 32, r0 : r0 + 32],
                    )
                    nc.scalar.copy(
                        bg[r0 : r0 + 32, j : j + 1],
                        pacc[g][r0 : r0 + 32, P : P + 1],
                    )
                amat.append(ag)
                bmat.append(bg)

        # ---------------- phase 2: queries ----------------
        with (
            tc.tile_pool(name="ps2n", bufs=5, space="PSUM") as ps2n,
            tc.tile_pool(name="ps2d", bufs=2, space="PSUM") as ps2d,
        ):
            _phase2(nc, tc, qt, out, io, ew, small, outp, ps2n, ps2d, amat, bmat)


def _phase2(nc, tc, qt, out, io, ew, small, outp, ps2n, ps2d, amat, bmat):
        for mq in range(N_QMACRO):
            c0 = mq * QMACRO * P
            phis = []
            for g in range(G):
                qt_t = io.tile([P, QMACRO * P], F32, tag=f"qt{g}", name=f"qt{g}")
                nc.sync.dma_start(
                    qt_t[:], qt[g * P : (g + 1) * P, c0 : c0 + QMACRO * P]
                )
                phis.append(_phi(nc, ew, qt_t, QMACRO * P))

            # den PSUM for the whole macro: cols (sub, g, j)
            den_ps = ps2d.tile([P, QMACRO * G * 4], F32, tag="den_ps")
            numers = []
            nm = None
            for i in range(QMACRO):
                if i % 2 == 0:
                    nm = ps2n.tile([P, 2 * HD], F32, tag="nm")
                    numers.append(nm)
                for g in range(G):
                    w = phis[g][:, i * P : (i + 1) * P]
                    nc.tensor.matmul(
                        nm[:, (i % 2) * HD + g * P : (i % 2) * HD + (g + 1) * P],
                        w,
                        amat[g][:],
                        start=True,
                        stop=True,
                    )
                    nc.tensor.matmul(
                        den_ps[:, (i * G + g) * 4 : (i * G + g + 1) * 4],
                        w,
                        bmat[g][:],
                        start=True,
                        stop=True,
                    )

            den_sb = small.tile([P, QMACRO * G * 4], F32, tag="den_sb")
            rcp = small.tile([P, QMACRO * G * 4], F32, tag="rcp")
            nc.vector.tensor_scalar(den_sb[:], den_ps[:], EPS, None, OP.add)
            nc.vector.reciprocal(rcp[:], den_sb[:])

            for pr in range(QMACRO // 2):
                out_t = outp.tile([P, 2 * HD], F32, tag="out_t")
                # rcp view for subs (2pr, 2pr+1): dims (s, g, j) + bcast v
                rv = rcp[:, 2 * pr * G * 4 : (2 * pr + 2) * G * 4].rearrange(
                    "p (s g j) -> p s g j", s=2, g=G
                )
                nc.vector.tensor_tensor(
                    out_t[:].rearrange("p (s g j c) -> p s g j c", s=2, g=G, c=32),
                    numers[pr][:].rearrange("p (s g j c) -> p s g j c", s=2, g=G, c=32),
                    _bcast_last(rv, 32),
                    OP.mult,
                )
                r0 = c0 + 2 * pr * P
                nc.sync.dma_start(
                    out[r0 : r0 + 2 * P, :].rearrange("(s p) c -> p s c", p=P),
                    out_t[:].rearrange("p (s c) -> p s c", s=2),
                )


_NC_CACHE = None


def build_nc():
    global _NC_CACHE
    if _NC_CACHE is not None:
        return _NC_CACHE
    nc = bacc.Bacc(
        "TRN2",
        target_bir_lowering=False,
        debug=False,
        enable_asserts=False,
        num_devices=N_BATCH,
    )
    qt = nc.dram_tensor("qt", [HD, L], F32, kind="ExternalInput").ap()
    kk = nc.dram_tensor("kk", [S, HD], F32, kind="ExternalInput").ap()
    vv = nc.dram_tensor("vv", [S, HD], F32, kind="ExternalInput").ap()
    out = nc.dram_tensor("out", [L, HD], F32, kind="ExternalOutput").ap()
    with tile.TileContext(nc) as tc:
        _build_body(nc, tc, qt, kk, vv, out)
    nc.compile()
    return nc


def make_in_maps(queries, keys, values):
    queries = np.asarray(queries, dtype=np.float32)
    keys = np.asarray(keys, dtype=np.float32)
    values = np.asarray(values, dtype=np.float32)
    in_maps = []
    for n in range(N_BATCH):
        qt = np.ascontiguousarray(
            queries[n].transpose(1, 2, 0).reshape(HD, L)
        )  # [h*32+d, l]
        in_maps.append(
            {
                "qt": qt,
                "kk": np.ascontiguousarray(keys[n].reshape(S, HD)),
                "vv": np.ascontiguousarray(values[n].reshape(S, HD)),
            }
        )
    return in_maps


def run(queries, keys, values, trace=False, **kwargs):
    nc = build_nc()
    in_maps = make_in_maps(queries, keys, values)
    res = run_bass_kernel_spmd(
        nc, in_maps, core_ids=list(range(N_BATCH)), trace=trace, **kwargs
    )
    outs = [res.results[n]["out"].reshape(L, H, D) for n in range(N_BATCH)]
    return np.stack(outs, axis=0), res


def kernel(queries, keys, values):
    out, _ = run(queries, keys, values, trace=False)
    return out


# revision 8
# speedup vs baseline: 4.6421x; 4.6421x over previous
"""Linear attention ("Transformers are RNNs") on 8 Trainium2 NeuronCores.

Problem: N=8, L=S=8192, H=8, D=Dv=32, f32.
    phi(x) = elu(x)+1
    A[d,v] = sum_s phi(K)[s,d] V[s,v]        (the /v_length ... *v_length cancels exactly)
    b[d]   = sum_s phi(K)[s,d]
    out[l,v] = (sum_d phi(Q)[l,d] A[d,v]) / (sum_d phi(Q)[l,d] b[d] + EPS)

Sharding: batch element n -> core n (fully independent, no collectives).

Device design (v2):
  - Q pre-transposed on host to [H*D, L]: contraction dim d on SBUF
    partitions, contiguous DMA, no on-device transposes.
  - phi(x) = min(exp(x), 1 + relu(x))  (exactly elu(x)+1), split across
    engines:  e = Exp(x) bf16 (ScalarE);  t = (x max 0)+1 bf16 (VectorE
    dual-op tensor_scalar, 2x);  phi = min(e, t) bf16 (VectorE tt, 2x mode).
  - Phase 1 (64 s-subtiles of 128): per 4-head group g one bf16 matmul
        lhsT = phi(K)[:, g]  [s=128, (j,d)=128]
        rhs  = [V_g | ones]  [s=128, 129]
    accumulated in PSUM[128,129] per group.  Diagonal 32x32 j-blocks of
    cols 0:128 are A_h; col 128 is b_h.
  - Phase 1.5: assemble per group: block-diag A  [128,128] bf16 and
    block-diag b columns [128,4] bf16.
  - Phase 2 (64 l-subtiles): per group two matmuls sharing the same
    stationary phiQ^T slice: numer (N=128, lands directly in output
    layout) and den (N=4, batched per 8-subtile macro in one PSUM bank so
    the eps-add + reciprocal amortize).  Normalize with one broadcast
    tensor_tensor multiply per 2 subtiles, DMA out [l, h*32+v] f32.
"""

import sys

for _p in ("/opt/trn_rl_repo",):
    if _p not in sys.path:
        sys.path.insert(0, _p)

import numpy as np

from concourse import bacc, bass, mybir, tile
from concourse.bass_utils import run_bass_kernel_spmd

# ---------------------------------------------------------------- constants
N_BATCH = 8
L = 8192
S = 8192
H = 8
D = 32
HD = H * D  # 256
P = 128
EPS = 1e-6

F32 = mybir.dt.float32
BF16 = mybir.dt.bfloat16
AF = mybir.ActivationFunctionType
OP = mybir.AluOpType

MACRO = 8  # 128-row s-subtiles per phase-1 macro tile
N_MACRO = S // (P * MACRO)  # 8
QMACRO = 8  # l-subtiles per phase-2 macro
N_QMACRO = L // (P * QMACRO)  # 8

G = 2  # head groups (4 heads each)
VA = P + 1  # 129: V group columns + ones column


def _bcast_last(ap, n):
    """Append a stride-0 dim of size n to an AP (free-dim broadcast)."""
    ap = ap.unsqueeze(ap.ndim)
    return ap.broadcast_to(tuple(ap.shape[:-1]) + (n,))


def _phi(nc, pool, x, fd):
    """phi(x) = elu(x)+1 = min(exp(x), 1 + relu(x)); x is [P, fd] f32 SBUF.

    Returns bf16. Engine split: ACT does exp, GpSimd the relu+1, DVE the min."""
    e = pool.tile([P, fd], BF16, tag="phi_e")
    t = pool.tile([P, fd], BF16, tag="phi_t")
    phi = pool.tile([P, fd], BF16, tag="phi_o")
    nc.scalar.activation(e[:], x[:], AF.Exp)
    nc.vector.tensor_scalar(t[:], x[:], 0.0, 1.0, OP.max, OP.add)
    nc.vector.tensor_tensor(phi[:], e[:], t[:], OP.min)
    return phi


def _build_body(nc, tc, qt, kk, vv, out):
    with (
        tc.tile_pool(name="io", bufs=3) as io,
        tc.tile_pool(name="ew", bufs=2) as ew,
        tc.tile_pool(name="misc", bufs=1) as misc,
        tc.tile_pool(name="small", bufs=3) as small,
        tc.tile_pool(name="outp", bufs=3) as outp,
    ):
        # ---------------- phase 1: A/b accumulation over S ----------------
        with tc.tile_pool(name="ps1", bufs=1, space="PSUM") as ps1:
            pacc = [
                ps1.tile([P, VA], F32, tag=f"pacc{g}", name=f"pacc{g}")
                for g in range(G)
            ]

            for m in range(N_MACRO):
                k_t = io.tile([P, MACRO * HD], F32, tag="k_t")
                v_t = io.tile([P, MACRO * HD], F32, tag="v_t")
                rows = slice(m * MACRO * P, (m + 1) * MACRO * P)
                nc.sync.dma_start(
                    k_t[:].rearrange("p (b c) -> p b c", b=MACRO),
                    kk[rows, :].rearrange("(b p) c -> p b c", p=P),
                )
                nc.sync.dma_start(
                    v_t[:].rearrange("p (b c) -> p b c", b=MACRO),
                    vv[rows, :].rearrange("(b p) c -> p b c", p=P),
                )
                # v_r: per (subtile, group): [V_g | ones], bf16
                v_r = io.tile([P, MACRO * G * VA], BF16, tag="v_r")
                v4 = v_r[:].rearrange("p (b g c) -> p b g c", b=MACRO, g=G)
                nc.scalar.copy(
                    v4[:, :, :, 0:P],
                    v_t[:].rearrange("p (b g c) -> p b g c", b=MACRO, g=G),
                )
                nc.vector.memset(v4[:, :, :, P : P + 1], 1.0)

                phi = _phi(nc, ew, k_t, MACRO * HD)

                for b in range(MACRO):
                    for g in range(G):
                        nc.tensor.matmul(
                            pacc[g][:],
                            phi[:, b * HD + g * P : b * HD + (g + 1) * P],
                            v_r[:, (b * G + g) * VA : (b * G + g + 1) * VA],
                            start=(m == 0 and b == 0),
                            stop=(m == N_MACRO - 1 and b == MACRO - 1),
                        )

            # ------------- phase 1.5: block-diag A, block-diag b ----------
            amat = []
            bmat = []
            for g in range(G):
                ag = misc.tile([P, P], BF16, tag=f"amat{g}", name=f"amat{g}")
                bg = misc.tile([P, 4], BF16, tag=f"bmat{g}", name=f"bmat{g}")
                nc.vector.memset(ag[:], 0.0)
                nc.vector.memset(bg[:], 0.0)
                for j in range(4):
                    r0 = 32 * j
                    nc.scalar.copy(
                        ag[r0 : r0 + 32, r0 : r0 + 32],
                        pacc[g][r0 : r0 + 32, r0 : r0 + 32],
                    )
                    nc.scalar.copy(
                        bg[r0 : r0 + 32, j : j + 1],
                        pacc[g][r0 : r0 + 32, P : P + 1],
                    )
                amat.append(ag)
                bmat.append(bg)

        # ---------------- phase 2: queries ----------------
        with (
            tc.tile_pool(name="ps2n", bufs=5, space="PSUM") as ps2n,
            tc.tile_pool(name="ps2d", bufs=2, space="PSUM") as ps2d,
        ):
            _phase2(nc, tc, qt, out, io, ew, small, outp, ps2n, ps2d, amat, bmat)


def _phase2(nc, tc, qt, out, io, ew, small, outp, ps2n, ps2d, amat, bmat):
        for mq in range(N_QMACRO):
            c0 = mq * QMACRO * P
            phis = []
            for g in range(G):
                qt_t = io.tile([P, QMACRO * P], F32, tag=f"qt{g}", name=f"qt{g}")
                nc.sync.dma_start(
                    qt_t[:], qt[g * P : (g + 1) * P, c0 : c0 + QMACRO * P]
                )
                phis.append(_phi(nc, ew, qt_t, QMACRO * P))

            # den PSUM for the whole macro: cols (sub, g, j)
            den_ps = ps2d.tile([P, QMACRO * G * 4], F32, tag="den_ps")
            numers = []
            nm = None
            for i in range(QMACRO):
                if i % 2 == 0:
                    nm = ps2n.tile([P, 2 * HD], F32, tag="nm")
                    numers.append(nm)
                for g in range(G):
                    w = phis[g][:, i * P : (i + 1) * P]
                    nc.tensor.matmul(
                        nm[:, (i % 2) * HD + g * P : (i % 2) * HD + (g + 1) * P],
                        w,
                        amat[g][:],
                        start=True,
                        stop=True,
                    )
                    nc.tensor.matmul(
                        den_ps[:, (i * G + g) * 4 : (i * G + g + 1) * 4],
                        w,
                        bmat[g][:],
                        start=True,
                        stop=True,
                    )

            den_sb = small.tile([P, QMACRO * G * 4], F32, tag="den_sb")
            rcp = small.tile([P, QMACRO * G * 4], F32, tag="rcp")
            nc.vector.tensor_scalar(den_sb[:], den_ps[:], EPS, None, OP.add)
            nc.vector.reciprocal(rcp[:], den_sb[:])

            for pr in range(QMACRO // 2):
                out_t = outp.tile([P, 2 * HD], F32, tag="out_t")
                # rcp view for subs (2pr, 2pr+1): dims (s, g, j) + bcast v
                rv = rcp[:, 2 * pr * G * 4 : (2 * pr + 2) * G * 4].rearrange(
                    "p (s g j) -> p s g j", s=2, g=G
                )
                nc.vector.tensor_tensor(
                    out_t[:].rearrange("p (s g j c) -> p s g j c", s=2, g=G, c=32),
                    numers[pr][:].rearrange("p (s g j c) -> p s g j c", s=2, g=G, c=32),
                    _bcast_last(rv, 32),
                    OP.mult,
                )
                r0 = c0 + 2 * pr * P
                nc.sync.dma_start(
                    out[r0 : r0 + 2 * P, :].rearrange("(s p) c -> p s c", p=P),
                    out_t[:].rearrange("p (s c) -> p s c", s=2),
                )


_NC_CACHE = None


def build_nc():
    global _NC_CACHE
    if _NC_CACHE is not None:
        return _NC_CACHE
    nc = bacc.Bacc(
        "TRN2",
        target_bir_lowering=False,
        debug=False,
        enable_asserts=False,
        num_devices=N_BATCH,
    )
    qt = nc.dram_tensor("qt", [HD, L], F32, kind="ExternalInput").ap()
    kk = nc.dram_tensor("kk", [S, HD], F32, kind="ExternalInput").ap()
    vv = nc.dram_tensor("vv", [S, HD], F32, kind="ExternalInput").ap()
    out = nc.dram_tensor("out", [L, HD], F32, kind="ExternalOutput").ap()
    with tile.TileContext(nc) as tc:
        _build_body(nc, tc, qt, kk, vv, out)
    nc.compile()
    return nc


def make_in_maps(queries, keys, values):
    queries = np.asarray(queries, dtype=np.float32)
    keys = np.asarray(keys, dtype=np.float32)
    values = np.asarray(values, dtype=np.float32)
    in_maps = []
    for n in range(N_BATCH):
        qt = np.ascontiguousarray(
            queries[n].transpose(1, 2, 0).reshape(HD, L)
        )  # [h*32+d, l]
        in_maps.append(
            {
                "qt": qt,
                "kk": np.ascontiguousarray(keys[n].reshape(S, HD)),
                "vv": np.ascontiguousarray(values[n].reshape(S, HD)),
            }
        )
    return in_maps


def run(queries, keys, values, trace=False, **kwargs):
    nc = build_nc()
    in_maps = make_in_maps(queries, keys, values)
    res = run_bass_kernel_spmd(
        nc, in_maps, core_ids=list(range(N_BATCH)), trace=trace, **kwargs
    )
    outs = [res.results[n]["out"].reshape(L, H, D) for n in range(N_BATCH)]
    return np.stack(outs, axis=0), res


def kernel(queries, keys, values):
    out, _ = run(queries, keys, values, trace=False)
    return out


# revision 10
# speedup vs baseline: 5.5423x; 1.1939x over previous
"""Linear attention ("Transformers are RNNs") on 8 Trainium2 NeuronCores.

Problem: N=8, L=S=8192, H=8, D=Dv=32, f32.
    phi(x) = elu(x)+1
    A[d,v] = sum_s phi(K)[s,d] V[s,v]        (the /v_length ... *v_length cancels exactly)
    b[d]   = sum_s phi(K)[s,d]
    out[l,v] = (sum_d phi(Q)[l,d] A[d,v]) / (sum_d phi(Q)[l,d] b[d] + EPS)

Sharding: batch element n -> core n (fully independent, no collectives).

Device design (v4):
  - bf16 compute throughout (rel err ~4e-3, gate is 2e-2): inputs are
    cast to bf16 on the host, halving DMA traffic; PSUM accumulation and
    the denominator/normalization stay f32.
  - Q pre-transposed on host to [H*D, L]: contraction dim d on SBUF
    partitions, contiguous DMA, no on-device transposes.
  - phi(x) = min(exp(x), 1 + relu(x))  (exactly elu(x)+1):
    e = Exp(x) (ScalarE); t = (x max 0)+1 (VectorE dual-op tensor_scalar,
    4x mode); phi = min(e, t) (VectorE tt, 2x mode).
  - Phase 1 (64 s-subtiles of 128): per 4-head group g one bf16 matmul
        lhsT = phi(K)[:, g]  [s=128, (j,d)=128]
        rhs  = [V_g | ones]  [s=128, 129]
    accumulated in PSUM[128,129] per group.  Diagonal 32x32 j-blocks of
    cols 0:128 are A_h; col 128 is b_h.
  - Phase 1.5: assemble per group: block-diag A [128,128] bf16 and
    block-diag b columns [128,4] bf16.
  - Phase 2 (64 l-subtiles): per group two matmuls sharing the same
    stationary phiQ^T slice: numer (N=128, lands directly in output
    layout) and den (N=4, batched per 8-subtile macro in one PSUM bank so
    the eps-add + reciprocal amortize).  Normalize with one broadcast
    tensor_tensor multiply per 2 subtiles; bf16 out, host casts to f32.
"""

import sys

for _p in ("/opt/trn_rl_repo",):
    if _p not in sys.path:
        sys.path.insert(0, _p)

import ml_dtypes
import numpy as np

from concourse import bacc, bass, mybir, tile
from concourse.bass_utils import run_bass_kernel_spmd

# ---------------------------------------------------------------- constants
N_BATCH = 8
L = 8192
S = 8192
H = 8
D = 32
HD = H * D  # 256
P = 128
EPS = 1e-6

F32 = mybir.dt.float32
BF16 = mybir.dt.bfloat16
AF = mybir.ActivationFunctionType
OP = mybir.AluOpType

MACRO = 8  # 128-row s-subtiles per phase-1 macro tile
N_MACRO = S // (P * MACRO)  # 8
QMACRO = 8  # l-subtiles per phase-2 macro
N_QMACRO = L // (P * QMACRO)  # 8

G = 2  # head groups (4 heads each)
VA = P + 1  # 129: V group columns + ones column


def _bcast_last(ap, n):
    """Append a stride-0 dim of size n to an AP (free-dim broadcast)."""
    ap = ap.unsqueeze(ap.ndim)
    return ap.broadcast_to(tuple(ap.shape[:-1]) + (n,))


def _phi(nc, pool, x, fd):
    """phi(x) = elu(x)+1 = min(exp(x), 1 + relu(x)); x is [P, fd] bf16 SBUF."""
    e = pool.tile([P, fd], BF16, tag="phi_e")
    t = pool.tile([P, fd], BF16, tag="phi_t")
    phi = pool.tile([P, fd], BF16, tag="phi_o")
    nc.scalar.activation(e[:], x[:], AF.Exp)
    nc.vector.tensor_scalar(t[:], x[:], 0.0, 1.0, OP.max, OP.add)
    nc.vector.tensor_tensor(phi[:], e[:], t[:], OP.min)
    return phi


def _build_body(nc, tc, qt, kk, vv, out):
    with (
        tc.tile_pool(name="io", bufs=4) as io,
        tc.tile_pool(name="ew", bufs=3) as ew,
        tc.tile_pool(name="misc", bufs=1) as misc,
        tc.tile_pool(name="small", bufs=3) as small,
        tc.tile_pool(name="outp", bufs=4) as outp,
    ):
        # ---------------- phase 1: A/b accumulation over S ----------------
        with tc.tile_pool(name="ps1", bufs=1, space="PSUM") as ps1:
            pacc = [
                ps1.tile([P, VA], F32, tag=f"pacc{g}", name=f"pacc{g}")
                for g in range(G)
            ]

            for m in range(N_MACRO):
                k_t = io.tile([P, MACRO * HD], BF16, tag="k_t")
                rows = slice(m * MACRO * P, (m + 1) * MACRO * P)
                nc.sync.dma_start(
                    k_t[:].rearrange("p (b c) -> p b c", b=MACRO),
                    kk[rows, :].rearrange("(b p) c -> p b c", p=P),
                )
                # v_r: per (subtile, group): [V_g | ones], bf16
                v_r = io.tile([P, MACRO * G * VA], BF16, tag="v_r")
                v4 = v_r[:].rearrange("p (b g c) -> p b g c", b=MACRO, g=G)
                for g in range(G):
                    nc.sync.dma_start(
                        v4[:, :, g, 0:P],
                        vv[rows, g * P : (g + 1) * P].rearrange(
                            "(b p) c -> p b c", p=P
                        ),
                    )
                nc.vector.memset(v4[:, :, :, P : P + 1], 1.0)

                phi = _phi(nc, ew, k_t, MACRO * HD)

                for b in range(MACRO):
                    for g in range(G):
                        nc.tensor.matmul(
                            pacc[g][:],
                            phi[:, b * HD + g * P : b * HD + (g + 1) * P],
                            v_r[:, (b * G + g) * VA : (b * G + g + 1) * VA],
                            start=(m == 0 and b == 0),
                            stop=(m == N_MACRO - 1 and b == MACRO - 1),
                        )

            # ------------- phase 1.5: block-diag A, block-diag b ----------
            amat = []
            bmat = []
            for g in range(G):
                ag = misc.tile([P, P], BF16, tag=f"amat{g}", name=f"amat{g}")
                bg = misc.tile([P, 4], BF16, tag=f"bmat{g}", name=f"bmat{g}")
                nc.vector.memset(ag[:], 0.0)
                nc.vector.memset(bg[:], 0.0)
                for j in range(4):
                    r0 = 32 * j
                    nc.scalar.copy(
                        ag[r0 : r0 + 32, r0 : r0 + 32],
                        pacc[g][r0 : r0 + 32, r0 : r0 + 32],
                    )
                    nc.scalar.copy(
                        bg[r0 : r0 + 32, j : j + 1],
                        pacc[g][r0 : r0 + 32, P : P + 1],
                    )
                amat.append(ag)
                bmat.append(bg)

        # ---------------- phase 2: queries ----------------
        with (
            tc.tile_pool(name="ps2n", bufs=5, space="PSUM") as ps2n,
            tc.tile_pool(name="ps2d", bufs=2, space="PSUM") as ps2d,
        ):
            for mq in range(N_QMACRO):
                c0 = mq * QMACRO * P
                phis = []
                for g in range(G):
                    qt_t = io.tile(
                        [P, QMACRO * P], BF16, tag=f"qt{g}", name=f"qt{g}"
                    )
                    nc.sync.dma_start(
                        qt_t[:], qt[g * P : (g + 1) * P, c0 : c0 + QMACRO * P]
                    )
                    phis.append(_phi(nc, ew, qt_t, QMACRO * P))

                # den PSUM for the whole macro: cols (sub, g, j)
                den_ps = ps2d.tile([P, QMACRO * G * 4], F32, tag="den_ps")
                numers = []
                nm = None
                for i in range(QMACRO):
                    if i % 2 == 0:
                        nm = ps2n.tile([P, 2 * HD], F32, tag="nm")
                        numers.append(nm)
                    for g in range(G):
                        w = phis[g][:, i * P : (i + 1) * P]
                        nc.tensor.matmul(
                            nm[:, (i % 2) * HD + g * P : (i % 2) * HD + (g + 1) * P],
                            w,
                            amat[g][:],
                            start=True,
                            stop=True,
                        )
                        nc.tensor.matmul(
                            den_ps[:, (i * G + g) * 4 : (i * G + g + 1) * 4],
                            w,
                            bmat[g][:],
                            start=True,
                            stop=True,
                        )

                den_sb = small.tile([P, QMACRO * G * 4], F32, tag="den_sb")
                rcp = small.tile([P, QMACRO * G * 4], F32, tag="rcp")
                nc.vector.tensor_scalar(den_sb[:], den_ps[:], EPS, None, OP.add)
                nc.vector.reciprocal(rcp[:], den_sb[:])

                for pr in range(QMACRO // 2):
                    out_t = outp.tile([P, 2 * HD], BF16, tag="out_t")
                    rv = rcp[:, 2 * pr * G * 4 : (2 * pr + 2) * G * 4].rearrange(
                        "p (s g j) -> p s g j", s=2, g=G
                    )
                    nc.vector.tensor_tensor(
                        out_t[:].rearrange(
                            "p (s g j c) -> p s g j c", s=2, g=G, c=32
                        ),
                        numers[pr][:].rearrange(
                            "p (s g j c) -> p s g j c", s=2, g=G, c=32
                        ),
                        _bcast_last(rv, 32),
                        OP.mult,
                    )
                    r0 = c0 + 2 * pr * P
                    nc.sync.dma_start(
                        out[r0 : r0 + 2 * P, :].rearrange("(s p) c -> p s c", p=P),
                        out_t[:].rearrange("p (s c) -> p s c", s=2),
                    )


_NC_CACHE = None


def build_nc():
    global _NC_CACHE
    if _NC_CACHE is not None:
        return _NC_CACHE
    nc = bacc.Bacc(
        "TRN2",
        target_bir_lowering=False,
        debug=False,
        enable_asserts=False,
        num_devices=N_BATCH,
    )
    qt = nc.dram_tensor("qt", [HD, L], BF16, kind="ExternalInput").ap()
    kk = nc.dram_tensor("kk", [S, HD], BF16, kind="ExternalInput").ap()
    vv = nc.dram_tensor("vv", [S, HD], BF16, kind="ExternalInput").ap()
    out = nc.dram_tensor("out", [L, HD], BF16, kind="ExternalOutput").ap()
    with tile.TileContext(nc) as tc:
        _build_body(nc, tc, qt, kk, vv, out)
    nc.compile()
    return nc


def make_in_maps(queries, keys, values):
    queries = np.asarray(queries, dtype=np.float32)
    keys = np.asarray(keys, dtype=np.float32)
    values = np.asarray(values, dtype=np.float32)
    bf = ml_dtypes.bfloat16
    in_maps = []
    for n in range(N_BATCH):
        qt = np.ascontiguousarray(
            queries[n].transpose(1, 2, 0).reshape(HD, L).astype(bf)
        )  # [h*32+d, l]
        in_maps.append(
            {
                "qt": qt,
                "kk": np.ascontiguousarray(keys[n].reshape(S, HD).astype(bf)),
                "vv": np.ascontiguousarray(values[n].reshape(S, HD).astype(bf)),
            }
        )
    return in_maps


def run(queries, keys, values, trace=False, **kwargs):
    nc = build_nc()
    in_maps = make_in_maps(queries, keys, values)
    res = run_bass_kernel_spmd(
        nc, in_maps, core_ids=list(range(N_BATCH)), trace=trace, **kwargs
    )
    outs = [
        res.results[n]["out"].astype(np.float32).reshape(L, H, D)
        for n in range(N_BATCH)
    ]
    return np.stack(outs, axis=0), res


def kernel(queries, keys, values):
    out, _ = run(queries, keys, values, trace=False)
    return out
